# revision 36
# baseline (speedup 1.0000x reference)
"""Trainium2 Bass kernel: varlen batched cross-attention (sparse_attention).

Math (per reference):
  qh = q @ Wq.T           [Tq, H, D]
  k,v = split(x @ Wkv.T)  [B, N, H, D]
  per batch b: queries of segment b attend over batch b's N keys
  out = softmax(qh k^T / sqrt(D)) v  -> [Tq, C] @ Wproj.T + bproj

Sharding: batch-parallel over 8 cores (core b owns batch b), zero
collectives. Host pre-transposes all operands so every device matmul
contracts over the partition axis. All queries padded to a uniform L
(multiple of 128) so one NEFF serves all cores.

Device layout (per core):
  xT [C, N], qT [C, L] bf16  (feature-major)
  K^T computed as head-pair tiles kt[hp] [128, N]  (d on partitions)
  S^T = K^T_h . qhT_h  per 128-key tile -> exp on ScalarE (scale fused)
  O^T + Z via V-augmented (ones col) matmuls, col-paired heads
  normalize with 1/Z broadcast through a tiny PE matmul
  y^T = Wproj^T . O^T + bias -> DMA out [C, L] f32; host transposes back
"""

import os
import numpy as np

B, NKEY, C, H, D = 8, 2048, 512, 8, 64
NCORES = 8
CT = C // 128          # 4 c-tiles
NT = NKEY // 128       # 16 key tiles
HPAIRS = H // 2        # 4 head pairs
SCALE = float(D) ** -0.5

_BUILD_CACHE = {}


def _lchunks(L):
    out = []
    off = 0
    while off < L:
        sz = min(512, L - off)
        out.append((off, sz))
        off += sz
    return out


def _build(L, dbg=False):
    key = (L, dbg)
    if key in _BUILD_CACHE:
        return _BUILD_CACHE[key]
    from contextlib import ExitStack
    import concourse.bass as bass
    import concourse.tile as tile
    import concourse.mybir as mybir
    from concourse import bacc

    f32 = mybir.dt.float32
    bf16 = mybir.dt.bfloat16
    AF = mybir.ActivationFunctionType
    ALU = mybir.AluOpType

    lch = _lchunks(L)

    nc = bacc.Bacc("TRN2", target_bir_lowering=False, debug=False)
    xT = nc.declare_dram_parameter("xT", [C, NKEY], bf16, isOutput=False)
    qT = nc.declare_dram_parameter("qT", [C, L], bf16, isOutput=False)
    wqT = nc.declare_dram_parameter("wqT", [C, C], bf16, isOutput=False)
    wkT = nc.declare_dram_parameter("wkT", [C, C], bf16, isOutput=False)
    wvT = nc.declare_dram_parameter("wvT", [C, C], bf16, isOutput=False)
    wpT = nc.declare_dram_parameter("wpT", [C, C], bf16, isOutput=False)
    biasP = nc.declare_dram_parameter("biasP", [128, CT], f32, isOutput=False)
    outT = nc.declare_dram_parameter("out", [C, L], f32, isOutput=True)
    if dbg:
        dbg_kt = nc.declare_dram_parameter("dbg_kt", [128, NKEY], f32, isOutput=True)
        dbg_qht = nc.declare_dram_parameter("dbg_qht", [128, L], f32, isOutput=True)
        dbg_pt = nc.declare_dram_parameter("dbg_pt", [128, 1024], f32, isOutput=True)
        dbg_poz = nc.declare_dram_parameter("dbg_poz", [128, 1024], f32, isOutput=True)
        dbg_onrm = nc.declare_dram_parameter("dbg_onrm", [128, 512], f32, isOutput=True)

    with ExitStack() as ctx:
        tc = ctx.enter_context(tile.TileContext(nc))
        pers = ctx.enter_context(tc.tile_pool(name="pers", bufs=1))
        psS = ctx.enter_context(tc.tile_pool(name="psS", bufs=2, space="PSUM"))
        psOZ = ctx.enter_context(tc.tile_pool(name="psOZ", bufs=2, space="PSUM"))
        psP = ctx.enter_context(tc.tile_pool(name="psP", bufs=2, space="PSUM"))
        ptp = ctx.enter_context(tc.tile_pool(name="ptp", bufs=24))
        work = ctx.enter_context(tc.tile_pool(name="work", bufs=2))

        # ---- persistent inputs -------------------------------------------
        xt_sb = [pers.tile([128, NKEY], bf16, tag=f"xt{i}", name=f"xt{i}") for i in range(CT)]
        qt_sb = [pers.tile([128, L], bf16, tag=f"qt{i}", name=f"qt{i}") for i in range(CT)]
        wq_sb = [pers.tile([128, C], bf16, tag=f"wq{i}", name=f"wq{i}") for i in range(CT)]
        wk_sb = [pers.tile([128, C], bf16, tag=f"wk{i}", name=f"wk{i}") for i in range(CT)]
        wv_sb = [pers.tile([128, C], bf16, tag=f"wv{i}", name=f"wv{i}") for i in range(CT)]
        wp_sb = [pers.tile([128, C], bf16, tag=f"wp{i}", name=f"wp{i}") for i in range(CT)]
        for i in range(CT):
            sl = slice(128 * i, 128 * (i + 1))
            nc.sync.dma_start(xt_sb[i][:], xT[sl, :])
            nc.sync.dma_start(wk_sb[i][:], wkT[sl, :])
            nc.sync.dma_start(wv_sb[i][:], wvT[sl, :])
        for i in range(CT):
            sl = slice(128 * i, 128 * (i + 1))
            nc.sync.dma_start(wq_sb[i][:], wqT[sl, :])
            nc.sync.dma_start(qt_sb[i][:], qT[sl, :])
            nc.sync.dma_start(wp_sb[i][:], wpT[sl, :])
        bias_sb = pers.tile([128, CT], f32, tag="bias")
        nc.sync.dma_start(bias_sb[:], biasP[:])

        # all-ones matrix: matmul(lhsT=ones, rhs=zacc) broadcasts the
        # partition-colsum of zacc to every output partition in one shot
        ones_sb = pers.tile([128, 128], bf16, tag="ones")
        nc.vector.memset(ones_sb[:], 1.0)

        # ---- projections --------------------------------------------------
        kt_sb = [pers.tile([128, NKEY], bf16, tag=f"kt{i}", name=f"kt{i}") for i in range(HPAIRS)]
        qht_sb = [pers.tile([128, L], bf16, tag=f"qht{i}", name=f"qht{i}") for i in range(HPAIRS)]
        vaug_sb = [pers.tile([128, H * (D + 1)], bf16, tag=f"va{i}", name=f"va{i}") for i in range(NT)]

        def proj_kt(jt):
            for nch in range(NKEY // 512):
                ps = psP.tile([128, 512], f32, tag="psP")
                for ct in range(CT):
                    nc.tensor.matmul(
                        ps[:, 0:512],
                        lhsT=wk_sb[ct][:, 128 * jt:128 * (jt + 1)],
                        rhs=xt_sb[ct][:, 512 * nch:512 * (nch + 1)],
                        start=(ct == 0), stop=(ct == CT - 1))
                nc.vector.tensor_copy(
                    kt_sb[jt][:, 512 * nch:512 * (nch + 1)], ps[:, 0:512])

        def proj_qht(jt):
            for (lcs, lcn) in lch:
                ps = psP.tile([128, 512], f32, tag="psP")
                for ct in range(CT):
                    nc.tensor.matmul(
                        ps[:, 0:lcn],
                        lhsT=wq_sb[ct][:, 128 * jt:128 * (jt + 1)],
                        rhs=qt_sb[ct][:, lcs:lcs + lcn],
                        start=(ct == 0), stop=(ct == CT - 1))
                nc.vector.tensor_copy(qht_sb[jt][:, lcs:lcs + lcn], ps[:, 0:lcn])

        def proj_v(nt):
            ps = psP.tile([128, 512], f32, tag="psP")
            for ct in range(CT):
                nc.tensor.matmul(
                    ps[:, 0:512],
                    lhsT=xt_sb[ct][:, 128 * nt:128 * (nt + 1)],
                    rhs=wv_sb[ct][:, 0:C],
                    start=(ct == 0), stop=(ct == CT - 1))
            va3 = vaug_sb[nt][:].rearrange("p (h e) -> p h e", h=H)
            ps3 = ps[:, 0:512].rearrange("p (h d) -> p h d", h=H)
            nc.vector.tensor_copy(va3[:, :, 0:D], ps3[:, :, :])
            nc.vector.memset(va3[:, :, D:D + 1], 1.0)

        # ---- attention group: head pair hp, l-chunk lc -------------------
        def attn(lc_i, hp):
            lcs, lcn = lch[lc_i]
            h1, h2 = 2 * hp, 2 * hp + 1
            pts = []
            for nt in range(NT):
                ps = psS.tile([128, 1024], f32, tag="psS")
                nsl = slice(128 * nt, 128 * (nt + 1))
                nc.tensor.matmul(
                    ps[:, 0:lcn],
                    lhsT=kt_sb[hp][0:64, nsl],
                    rhs=qht_sb[hp][0:64, lcs:lcs + lcn],
                    start=True, stop=True, tile_position=(0, 0))
                nc.tensor.matmul(
                    ps[:, 512:512 + lcn],
                    lhsT=kt_sb[hp][64:128, nsl],
                    rhs=qht_sb[hp][64:128, lcs:lcs + lcn],
                    start=True, stop=True, tile_position=(64, 0))
                pt = ptp.tile([128, 1024], bf16, tag="pt")
                ps2 = ps[:, 0:1024].rearrange("p (b x) -> p b x", b=2)
                pt2 = pt[:, 0:1024].rearrange("p (b x) -> p b x", b=2)
                nc.scalar.activation(pt2[:, :, 0:lcn], ps2[:, :, 0:lcn],
                                     AF.Exp, scale=SCALE)
                pts.append(pt)
                if nt == 1:
                    zacc = work.tile([128, 1024], bf16, tag="zacc")
                    nc.vector.tensor_tensor(zacc[:, :], pts[0][:, 0:1024],
                                            pts[1][:, 0:1024], ALU.add)
                elif nt > 1:
                    nc.vector.tensor_tensor(zacc[:, :], zacc[:, :],
                                            pts[nt][:, 0:1024], ALU.add)
            # O^T accumulated over the 16 key tiles (col-paired heads).
            poz = psOZ.tile([128, 512], f32, tag="psOZ")
            nc.vector.memset(poz[:, :], 0.0)
            for nt in range(NT):
                va3 = vaug_sb[nt][:].rearrange("p (h e) -> p h e", h=H)
                stop = (nt == NT - 1)
                nc.tensor.matmul(
                    poz[0:64, 0:lcn], lhsT=va3[:, h1, 0:D],
                    rhs=pts[nt][:, 0:lcn],
                    start=False, stop=stop, tile_position=(0, 0),
                    skip_group_check=True)
                nc.tensor.matmul(
                    poz[64:128, 0:lcn], lhsT=va3[:, h2, 0:D],
                    rhs=pts[nt][:, 512:512 + lcn],
                    start=False, stop=stop, tile_position=(0, 64),
                    skip_group_check=True)
            # broadcast-sum Z to all partitions: ones^T . zacc
            pbz = psP.tile([128, 512], f32, tag="psP")
            nc.tensor.matmul(pbz[:, 0:lcn], lhsT=ones_sb[:, 0:128],
                             rhs=zacc[:, 0:lcn], start=True, stop=True)
            pbz2 = psP.tile([128, 512], f32, tag="psP")
            nc.tensor.matmul(pbz2[:, 0:lcn], lhsT=ones_sb[:, 0:128],
                             rhs=zacc[:, 512:512 + lcn], start=True, stop=True)
            if dbg and lc_i == 0 and hp == 0:
                dcp2 = work.tile([128, 1024], f32, tag="dcp2")
                nc.vector.tensor_copy(dcp2[:, :], poz[:, 0:1024])
                nc.sync.dma_start(dbg_poz[:, :], dcp2[:, :])
            bz_sb = work.tile([128, 1024], f32, tag="bz")
            nc.vector.reciprocal_approx_fast(bz_sb[:, 0:lcn], pbz[:, 0:lcn])
            nc.vector.reciprocal_approx_fast(bz_sb[:, 512:512 + lcn],
                                             pbz2[:, 0:lcn])
            onrm = work.tile([128, 512], bf16, tag=f"onrm{hp}")
            nc.vector.tensor_tensor(onrm[0:64, 0:lcn], poz[0:64, 0:lcn],
                                    bz_sb[0:64, 0:lcn], ALU.mult)
            nc.vector.tensor_tensor(onrm[64:128, 0:lcn], poz[64:128, 0:lcn],
                                    bz_sb[64:128, 512:512 + lcn], ALU.mult)
            if dbg and lc_i == 0 and hp == 0:
                dcp3 = work.tile([128, 512], f32, tag="dcp3")
                nc.vector.tensor_copy(dcp3[:, :], onrm[:, 0:512])
                nc.sync.dma_start(dbg_onrm[:, :], dcp3[:, :])
            return onrm

        def proj_out(lc_i, onrms):
            lcs, lcn = lch[lc_i]
            for jt in range(CT):
                py = psOZ.tile([128, 512], f32, tag="psOZ")
                ys = work.tile([128, 512], f32, tag="ys")
                for hp in range(HPAIRS):
                    nc.tensor.matmul(
                        py[:, 0:lcn],
                        lhsT=wp_sb[hp][:, 128 * jt:128 * (jt + 1)],
                        rhs=onrms[hp][:, 0:lcn],
                        start=(hp == 0), stop=(hp == HPAIRS - 1))
                nc.vector.tensor_scalar(
                    ys[:, 0:lcn], py[:, 0:lcn],
                    bias_sb[:, jt:jt + 1], None, ALU.add)
                nc.sync.dma_start(
                    outT[128 * jt:128 * (jt + 1), lcs:lcs + lcn],
                    ys[:, 0:lcn])

        # ---- emission order (scheduling priority) ------------------------
        proj_kt(0)
        proj_qht(0)
        proj_kt(1)
        proj_qht(1)
        # Remaining projections at background priority: they run in PE gaps
        # of the ACT-bound exp stream instead of serializing up front.
        with tc.high_priority(offset=-(10 ** 6)):
            for nt in range(NT):
                proj_v(nt)
            for jt in range(2, HPAIRS):
                proj_kt(jt)
                proj_qht(jt)
        if dbg:
            dk = work.tile([128, NKEY], f32, tag="dk", bufs=1)
            nc.vector.tensor_copy(dk[:, :], kt_sb[0][:, :])
            nc.sync.dma_start(dbg_kt[:, :], dk[:, :])
            dq = work.tile([128, L], f32, tag="dq", bufs=1)
            nc.vector.tensor_copy(dq[:, :], qht_sb[0][:, :])
            nc.sync.dma_start(dbg_qht[:, :], dq[:, :])
        pending = None
        for lc_i in range(len(lch)):
            onrms = []
            for hp in range(HPAIRS):
                onrms.append(attn(lc_i, hp))
                if hp == 0 and pending is not None:
                    proj_out(*pending)
                    pending = None
            pending = (lc_i, onrms)
        proj_out(*pending)

    nc.compile()
    _BUILD_CACHE[key] = nc
    return nc


def kernel(x, q, Wq, Wkv, Wproj, bproj, q_lengths, max_q_len):
    import ml_dtypes
    from concourse.bass_utils import run_bass_kernel_spmd

    bf16 = ml_dtypes.bfloat16
    x = np.asarray(x, np.float32)
    q = np.asarray(q, np.float32)
    Wq = np.asarray(Wq, np.float32)
    Wkv = np.asarray(Wkv, np.float32)
    Wproj = np.asarray(Wproj, np.float32)
    bproj = np.asarray(bproj, np.float32)
    q_lengths = np.asarray(q_lengths, np.int64)
    assert x.shape[0] == NCORES == B

    L = int(((q_lengths.max() + 127) // 128) * 128)
    nc = _build(L)

    offs = np.concatenate([[0], np.cumsum(q_lengths)])
    wqT = np.ascontiguousarray(Wq.T).astype(bf16)
    wkT = np.ascontiguousarray(Wkv[:C].T).astype(bf16)
    wvT = np.ascontiguousarray(Wkv[C:].T).astype(bf16)
    wpT = np.ascontiguousarray(Wproj.T).astype(bf16)
    biasP = np.ascontiguousarray(bproj.reshape(CT, 128).T).astype(np.float32)

    in_maps = []
    for b in range(B):
        Lb = int(q_lengths[b])
        qseg = q[offs[b]:offs[b] + Lb]
        qTp = np.zeros((C, L), bf16)
        qTp[:, :Lb] = qseg.T.astype(bf16)
        in_maps.append({
            "xT": np.ascontiguousarray(x[b].T).astype(bf16),
            "qT": qTp,
            "wqT": wqT, "wkT": wkT, "wvT": wvT, "wpT": wpT,
            "biasP": biasP,
        })

    trace = os.environ.get("KERNEL_TRACE", "") == "1"
    if trace:
        try:
            import sys
            import types
            import antenv
            if "antenv.axon_hooks" not in sys.modules:
                from trn_agent_boot.trn_boot import _ntff_profile_via_ctypes
                hook = _ntff_profile_via_ctypes("/opt/axon/libaxon_pjrt.so")
                mod = types.ModuleType("antenv.axon_hooks")
                mod.get_axon_ntff_profile_hook = lambda: hook
                sys.modules["antenv.axon_hooks"] = mod
                antenv.axon_hooks = mod
        except Exception as e:
            print(f"ntff hook setup failed: {e}")
            trace = False
    res = run_bass_kernel_spmd(nc, in_maps, core_ids=list(range(NCORES)),
                               trace=trace)
    if trace and res.exec_time_ns is not None:
        print(f"HW exec time: {res.exec_time_ns} ns")
        if res.instructions_and_trace:
            print(f"trace: {res.instructions_and_trace[1]}")

    out = np.empty((int(offs[-1]), C), np.float32)
    for b in range(B):
        Lb = int(q_lengths[b])
        out[offs[b]:offs[b] + Lb] = res.results[b]["out"][:, :Lb].T
    return out
